# revision 4
# baseline (speedup 1.0000x reference)
"""2-layer GCN on 8 Trainium2 NeuronCores.

Math (dense formulation):
    A~ = scatter_ones(edge_index) + I          (entries in {0,1,2}, exact in bf16)
    d  = clip(A~.sum(1), 1)^-1/2
    agg(H) = (d ⊙_row (A~ @ (d ⊙_row H)))      ("normalized aggregation")
    h   = relu(agg(x) @ W1 + b1)
    out = agg(h) @ W2 + b2

Sharding: rows of A~ (= output nodes) are split across 8 cores. Each core gets
A~.T[:, rows_i] in a partition-major layout and computes its row-slice of both
aggregations on the tensor engine (contraction over nodes on the partition
axis, so the aggregation output lands feature-major = exactly the lhsT layout
the following weight-matmul needs). The inner d-scaling is folded into x on
the host; the outer d-scaling + bias + relu run on DVE/ACT per 128-row block.
Between the layers the scaled hidden features hs = d ⊙ relu(...) are
AllGathered (bf16) so every core holds all nodes' features for the second
aggregation.

Perf structure (from neuron-profile iterations):
- all DRAM layouts partition-major so DMA lines are >=8KB (node-major layouts
  capped DMA at ~1-2KB lines and made it the co-bottleneck),
- contraction chunks are visited in kk-major order (chunk j <-> global chunk
  (j%8)*n_rb + j//8) so the hidden-feature AllGather can be split in two: the
  second half runs while the tensor engine aggregates the first half,
- the implicit kernel-entry barrier collective (~40us) is dropped; the
  mid-kernel AllGathers are the only cross-core synchronization.
"""

import sys

if '/opt/trn_rl_repo' not in sys.path:
    sys.path.insert(0, '/opt/trn_rl_repo')

import numpy as np
import ml_dtypes

import concourse.bass as bass
import concourse.tile as tile
from concourse import bacc, mybir
from concourse.bass_utils import run_bass_kernel_spmd

N_CORES = 8
BF16 = mybir.dt.bfloat16
F32 = mybir.dt.float32

# filled by kernel() on each run; test.py reads exec_time_ns from here
LAST_RESULT = None

_NC_CACHE = {}


def _k_order(n_k, n_rb):
    """kk-major visit order: j -> global chunk (j % N_CORES)*n_rb + j//N_CORES."""
    return [(j % N_CORES) * n_rb + (j // N_CORES) for j in range(n_k)]


def build_gcn(n_nodes, in_f, hid, out_f):
    rows = n_nodes // N_CORES     # output rows per core
    n_k = n_nodes // 128          # contraction chunks (global)
    n_rb = rows // 128            # 128-row blocks per core
    rw = min(512, rows)           # row free-dim chunk for aggregation matmuls
    n_rh = rows // rw
    n_fi = in_f // 128
    n_fh = hid // 128
    KB = min(4, n_k)              # k-chunks per AT stream DMA
    n_g = n_k // KB
    XC = min(16, n_k)             # k-chunks per resident-x chunk
    n_xc = n_k // XC
    half = n_rb // 2              # AllGather split point (0 -> no split)

    nc = bacc.Bacc(num_devices=N_CORES)

    at_ext = nc.declare_dram_parameter("at", [128, n_k * rows], BF16, isOutput=False)
    xs_ext = nc.declare_dram_parameter("xs", [128, n_k * in_f], BF16, isOutput=False)
    w1_ext = nc.declare_dram_parameter("w1", [in_f, hid], BF16, isOutput=False)
    w2_ext = nc.declare_dram_parameter("w2", [hid, out_f], BF16, isOutput=False)
    b1_ext = nc.declare_dram_parameter("b1bc", [128, hid], F32, isOutput=False)
    b2_ext = nc.declare_dram_parameter("b2bc", [128, out_f], F32, isOutput=False)
    dr_ext = nc.declare_dram_parameter("dr8", [128, n_rb], F32, isOutput=False)
    out_ext = nc.declare_dram_parameter("out", [rows, out_f], F32, isOutput=True)

    # hs in partition-major layout: [p, rb*hid + f] = hs[rb*128+p, f],
    # split into two tensors so each AllGather depends only on its half.
    n_splits = 2 if half > 0 else 1
    split_rbs = [list(range(half)), list(range(half, n_rb))] if n_splits == 2 \
        else [list(range(n_rb))]
    hs_loc = []
    hs_gath = []
    for s, rbs in enumerate(split_rbs):
        hs_loc.append(nc.dram_tensor(f"hs_loc{s}", [128, len(rbs) * hid], BF16))
        hs_gath.append(nc.dram_tensor(
            f"hs_gath{s}", [N_CORES * 128, len(rbs) * hid], BF16,
            addr_space="Shared"))

    with tile.TileContext(nc) as tc:
        with (
            tc.tile_pool(name="const", bufs=1) as const_pool,
            tc.tile_pool(name="stream", bufs=3) as stream,
            tc.tile_pool(name="xsrc", bufs=1) as xsrc,
            tc.tile_pool(name="feat", bufs=max(n_fi, n_fh)) as feat,
            tc.tile_pool(name="ep", bufs=2) as ep,
            tc.tile_pool(name="psum", bufs=8, space="PSUM") as psum,
        ):
            # first compute dependency: xs chunk 0 (sync queue, ahead of all)
            xsr = [xsrc.tile([128, XC * in_f], BF16, tag=f"xsr_{c}",
                             name=f"xsr_{c}") for c in range(n_xc)]
            nc.sync.dma_start(xsr[0][:], xs_ext[:, 0:XC * in_f])

            # constants on the gpsimd queue so they don't delay the stream
            w1t = []
            for fc in range(n_fi):
                t = const_pool.tile([128, hid], BF16, tag=f"w1_{fc}")
                nc.gpsimd.dma_start(t[:], w1_ext[fc * 128:(fc + 1) * 128, :])
                w1t.append(t)
            w2t = []
            for fc in range(n_fh):
                t = const_pool.tile([128, out_f], BF16, tag=f"w2_{fc}")
                nc.gpsimd.dma_start(t[:], w2_ext[fc * 128:(fc + 1) * 128, :])
                w2t.append(t)
            b1t = const_pool.tile([128, hid], F32, tag="b1")
            nc.gpsimd.dma_start(b1t[:], b1_ext[:])
            b2t = const_pool.tile([128, out_f], F32, tag="b2")
            nc.gpsimd.dma_start(b2t[:], b2_ext[:])
            drt = const_pool.tile([128, n_rb], F32, tag="dr")
            nc.gpsimd.dma_start(drt[:], dr_ext[:])

            for c in range(1, n_xc):
                nc.sync.dma_start(
                    xsr[c][:], xs_ext[:, c * XC * in_f:(c + 1) * XC * in_f]
                )

            def xs_slice(j, f):
                c, kk = j // XC, j % XC
                return xsr[c][:, kk * in_f + f * 128: kk * in_f + (f + 1) * 128]

            hsg = [[], []]  # [split][rank] tiles, filled after each AllGather

            def hs_slice(j, f):
                kk, i = j // N_CORES, j % N_CORES
                s = 0 if (n_splits == 1 or kk < half) else 1
                kk_s = kk if s == 0 else kk - half
                return hsg[s][i][:, kk_s * hid + f * 128: kk_s * hid + (f + 1) * 128]

            def aggregate(src_slice, n_f, label):
                """P_T[f, r] = sum_n src[n, f] * A~[r, n], feature-major psum."""
                acc = [
                    psum.tile([128, rw], F32, tag="acc", name=f"acc_{label}_{i}")
                    for i in range(n_f * n_rh)
                ]
                for g in range(n_g):
                    atq = stream.tile([128, KB * rows], BF16, tag="atq",
                                      name=f"atq_{label}_{g}")
                    nc.sync.dma_start(
                        atq[:], at_ext[:, g * KB * rows:(g + 1) * KB * rows]
                    )
                    for kk in range(KB):
                        j = g * KB + kk
                        for f in range(n_f):
                            for rh in range(n_rh):
                                nc.tensor.matmul(
                                    acc[f * n_rh + rh][:],
                                    src_slice(j, f),
                                    atq[:, kk * rows + rh * rw:
                                        kk * rows + (rh + 1) * rw],
                                    start=(j == 0),
                                    stop=(j == n_k - 1),
                                )
                # drain feature-major accumulation to SBUF (cast bf16)
                ps = []
                for f in range(n_f):
                    t = feat.tile([128, rows], BF16, tag="ps", name=f"ps_{label}_{f}")
                    for rh in range(n_rh):
                        nc.vector.tensor_copy(
                            t[:, rh * rw:(rh + 1) * rw], acc[f * n_rh + rh][:]
                        )
                    ps.append(t)
                return ps

            def fire_allgather(s):
                nc.gpsimd.collective_compute(
                    "AllGather",
                    mybir.AluOpType.bypass,
                    replica_groups=[list(range(N_CORES))],
                    ins=[hs_loc[s][:]],
                    outs=[hs_gath[s][:]],
                )
                w = len(split_rbs[s]) * hid
                for i in range(N_CORES):
                    t = xsrc.tile([128, w], BF16, tag=f"hsg_{s}_{i}",
                                  name=f"hsg_{s}_{i}")
                    nc.sync.dma_start(t[:], hs_gath[s][i * 128:(i + 1) * 128, :])
                    hsg[s].append(t)

            # ---- layer 1 ----
            p1s = aggregate(xs_slice, n_fi, "agg1")
            for rb in range(n_rb):
                zp = psum.tile([128, hid], F32, tag="acc")
                for fc in range(n_fi):
                    nc.tensor.matmul(
                        zp[:],
                        p1s[fc][:, rb * 128:(rb + 1) * 128],
                        w1t[fc][:],
                        start=(fc == 0),
                        stop=(fc == n_fi - 1),
                    )
                v = ep.tile([128, hid], F32, tag="v1")
                nc.vector.tensor_scalar_mul(v[:], zp[:], drt[:, rb:rb + 1])
                v2 = ep.tile([128, hid], F32, tag="v2")
                nc.vector.tensor_add(v2[:], v[:], b1t[:])
                hst = ep.tile([128, hid], BF16, tag="hst")
                nc.scalar.activation(
                    hst[:], v2[:], mybir.ActivationFunctionType.Relu,
                    scale=drt[:, rb:rb + 1],
                )
                s = 0 if (n_splits == 1 or rb < half) else 1
                rb_s = rb if s == 0 else rb - half
                nc.sync.dma_start(
                    hs_loc[s][:, rb_s * hid:(rb_s + 1) * hid], hst[:]
                )
                if n_splits == 2 and rb == half - 1:
                    fire_allgather(0)
            fire_allgather(1 if n_splits == 2 else 0)

            # ---- layer 2 ----
            p2s = aggregate(hs_slice, n_fh, "agg2")
            for rb in range(n_rb):
                zp = psum.tile([128, out_f], F32, tag="acc")
                for fc in range(n_fh):
                    nc.tensor.matmul(
                        zp[:],
                        p2s[fc][:, rb * 128:(rb + 1) * 128],
                        w2t[fc][:],
                        start=(fc == 0),
                        stop=(fc == n_fh - 1),
                    )
                v = ep.tile([128, out_f], F32, tag="vo1")
                nc.vector.tensor_scalar_mul(v[:], zp[:], drt[:, rb:rb + 1])
                o = ep.tile([128, out_f], F32, tag="vo2")
                nc.vector.tensor_add(o[:], v[:], b2t[:])
                nc.sync.dma_start(out_ext[rb * 128:(rb + 1) * 128, :], o[:])

    # drop the implicit kernel-entry barrier collective (~40us): the
    # mid-kernel AllGathers provide all the cross-core sync the math needs.
    nc._bir_kernel_barrier_sem_replica_groups = []
    nc.finalize()
    return nc


def _to_partition_major(a, n_k, order=None):
    """[n_k*128, F] row-major -> [128, n_k*F], chunk order[j] at column j*F."""
    f = a.shape[1]
    b = a.reshape(n_k, 128, f)
    if order is not None:
        b = b[order]
    return np.ascontiguousarray(b.transpose(1, 0, 2).reshape(128, n_k * f))


def prep_inputs(x, edge_index, W1, b1, W2, b2):
    """Host-side prep: dense normalized adjacency + per-core shards."""
    x = np.asarray(x, dtype=np.float32)
    edge_index = np.asarray(edge_index)
    W1 = np.asarray(W1, dtype=np.float32)
    b1 = np.asarray(b1, dtype=np.float32)
    W2 = np.asarray(W2, dtype=np.float32)
    b2 = np.asarray(b2, dtype=np.float32)

    n = x.shape[0]
    rows = n // N_CORES
    n_rb = rows // 128
    n_k = n // 128
    order = _k_order(n_k, n_rb)

    adj = np.zeros((n, n), dtype=np.float32)
    adj[edge_index[0], edge_index[1]] = 1.0
    idx = np.arange(n)
    adj[idx, idx] += 1.0
    deg = np.maximum(adj.sum(axis=1), 1.0)
    dinv = (deg ** -0.5).astype(np.float32)

    xs = _to_partition_major(
        (x * dinv[:, None]).astype(ml_dtypes.bfloat16), n_k, order
    )
    w1b = W1.astype(ml_dtypes.bfloat16)
    w2b = W2.astype(ml_dtypes.bfloat16)
    b1bc = np.ascontiguousarray(np.broadcast_to(b1, (128, b1.shape[0]))).astype(np.float32)
    b2bc = np.ascontiguousarray(np.broadcast_to(b2, (128, b2.shape[0]))).astype(np.float32)

    in_maps = []
    for i in range(N_CORES):
        sl = slice(i * rows, (i + 1) * rows)
        ati = np.ascontiguousarray(adj[sl, :].T).astype(ml_dtypes.bfloat16)
        in_maps.append({
            "at": _to_partition_major(ati, n_k, order),
            "xs": xs,
            "w1": w1b,
            "w2": w2b,
            "b1bc": b1bc,
            "b2bc": b2bc,
            "dr8": np.ascontiguousarray(dinv[sl].reshape(n_rb, 128).T),
        })
    return in_maps


def kernel(x, edge_index, W1, b1, W2, b2):
    global LAST_RESULT
    x = np.asarray(x)
    n, in_f = x.shape
    hid = np.asarray(W1).shape[1]
    out_f = np.asarray(W2).shape[1]

    key = (n, in_f, hid, out_f)
    if key not in _NC_CACHE:
        _NC_CACHE[key] = build_gcn(n, in_f, hid, out_f)
    nc = _NC_CACHE[key]

    in_maps = prep_inputs(x, edge_index, W1, b1, W2, b2)
    res = run_bass_kernel_spmd(nc, in_maps, core_ids=list(range(N_CORES)))
    LAST_RESULT = res
    return np.concatenate([res.results[i]["out"] for i in range(N_CORES)], axis=0)


# revision 5
# speedup vs baseline: 1.0575x; 1.0575x over previous
"""2-layer GCN on 8 Trainium2 NeuronCores.

Math (dense formulation):
    A~ = scatter_ones(edge_index) + I          (entries in {0,1,2}, exact in bf16)
    d  = clip(A~.sum(1), 1)^-1/2
    agg(H) = (d ⊙_row (A~ @ (d ⊙_row H)))      ("normalized aggregation")
    h   = relu(agg(x) @ W1 + b1)
    out = agg(h) @ W2 + b2

Sharding: rows of A~ (= output nodes) are split across 8 cores. Each core gets
A~.T[:, rows_i] in a partition-major layout and computes its row-slice of both
aggregations on the tensor engine (contraction over nodes on the partition
axis, so the aggregation output lands feature-major = exactly the lhsT layout
the following weight-matmul needs). The inner d-scaling is folded into x on
the host; the outer d-scaling + bias + relu run on DVE/ACT per 128-row block.
Between the layers the scaled hidden features hs = d ⊙ relu(...) are
AllGathered (bf16) so every core holds all nodes' features for the second
aggregation.

Perf structure (from neuron-profile iterations):
- all DRAM layouts partition-major so DMA lines are >=8KB (node-major layouts
  capped DMA at ~1-2KB lines and made it the co-bottleneck),
- contraction chunks are visited in kk-major order (chunk j <-> global chunk
  (j%8)*n_rb + j//8) so the hidden-feature AllGather can be split in two: the
  second half runs while the tensor engine aggregates the first half,
- the implicit kernel-entry barrier collective (~40us) is dropped; the
  mid-kernel AllGathers are the only cross-core synchronization.
"""

import sys

if '/opt/trn_rl_repo' not in sys.path:
    sys.path.insert(0, '/opt/trn_rl_repo')

import numpy as np
import ml_dtypes

import concourse.bass as bass
import concourse.tile as tile
from concourse import bacc, mybir
from concourse.bass_utils import run_bass_kernel_spmd

N_CORES = 8
BF16 = mybir.dt.bfloat16
F32 = mybir.dt.float32

# filled by kernel() on each run; test.py reads exec_time_ns from here
LAST_RESULT = None

_NC_CACHE = {}


def _k_order(n_k, n_rb):
    """kk-major visit order: j -> global chunk (j % N_CORES)*n_rb + j//N_CORES."""
    return [(j % N_CORES) * n_rb + (j // N_CORES) for j in range(n_k)]


def build_gcn(n_nodes, in_f, hid, out_f):
    rows = n_nodes // N_CORES     # output rows per core
    n_k = n_nodes // 128          # contraction chunks (global)
    n_rb = rows // 128            # 128-row blocks per core
    rw = min(512, rows)           # row free-dim chunk for aggregation matmuls
    n_rh = rows // rw
    n_fi = in_f // 128
    n_fh = hid // 128
    KB = min(4, n_k)              # k-chunks per AT stream DMA
    n_g = n_k // KB
    XC = min(16, n_k)             # k-chunks per resident-x chunk
    n_xc = n_k // XC
    half = n_rb // 2              # AllGather split point (0 -> no split)

    nc = bacc.Bacc(num_devices=N_CORES)

    at_ext = nc.declare_dram_parameter("at", [128, n_k * rows], BF16, isOutput=False)
    xs_ext = nc.declare_dram_parameter("xs", [128, n_k * in_f], BF16, isOutput=False)
    w1_ext = nc.declare_dram_parameter("w1", [in_f, hid], BF16, isOutput=False)
    w2_ext = nc.declare_dram_parameter("w2", [hid, out_f], BF16, isOutput=False)
    b1_ext = nc.declare_dram_parameter("b1bc", [128, hid], F32, isOutput=False)
    b2_ext = nc.declare_dram_parameter("b2bc", [128, out_f], F32, isOutput=False)
    dr_ext = nc.declare_dram_parameter("dr8", [128, n_rb], F32, isOutput=False)
    out_ext = nc.declare_dram_parameter("out", [rows, out_f], F32, isOutput=True)

    # hs in partition-major layout: [p, rb*hid + f] = hs[rb*128+p, f],
    # split into two tensors so each AllGather depends only on its half.
    n_splits = 1
    split_rbs = [list(range(half)), list(range(half, n_rb))] if n_splits == 2 \
        else [list(range(n_rb))]
    hs_loc = []
    hs_gath = []
    for s, rbs in enumerate(split_rbs):
        hs_loc.append(nc.dram_tensor(f"hs_loc{s}", [128, len(rbs) * hid], BF16))
        hs_gath.append(nc.dram_tensor(
            f"hs_gath{s}", [N_CORES * 128, len(rbs) * hid], BF16,
            addr_space="Shared"))

    with tile.TileContext(nc) as tc:
        with (
            tc.tile_pool(name="const", bufs=1) as const_pool,
            tc.tile_pool(name="stream", bufs=3) as stream,
            tc.tile_pool(name="xsrc", bufs=1) as xsrc,
            tc.tile_pool(name="hstream", bufs=6) as hstream,
            tc.tile_pool(name="feat", bufs=max(n_fi, n_fh)) as feat,
            tc.tile_pool(name="ep", bufs=2) as ep,
            tc.tile_pool(name="psum", bufs=8, space="PSUM") as psum,
        ):
            # first compute dependency: xs chunk 0 (sync queue, ahead of all)
            xsr = [xsrc.tile([128, XC * in_f], BF16, tag=f"xsr_{c}",
                             name=f"xsr_{c}") for c in range(n_xc)]
            nc.sync.dma_start(xsr[0][:], xs_ext[:, 0:XC * in_f])

            # constants on the gpsimd queue so they don't delay the stream
            w1t = []
            for fc in range(n_fi):
                t = const_pool.tile([128, hid], BF16, tag=f"w1_{fc}")
                nc.gpsimd.dma_start(t[:], w1_ext[fc * 128:(fc + 1) * 128, :])
                w1t.append(t)
            w2t = []
            for fc in range(n_fh):
                t = const_pool.tile([128, out_f], BF16, tag=f"w2_{fc}")
                nc.gpsimd.dma_start(t[:], w2_ext[fc * 128:(fc + 1) * 128, :])
                w2t.append(t)
            b1t = const_pool.tile([128, hid], F32, tag="b1")
            nc.gpsimd.dma_start(b1t[:], b1_ext[:])
            b2t = const_pool.tile([128, out_f], F32, tag="b2")
            nc.gpsimd.dma_start(b2t[:], b2_ext[:])
            drt = const_pool.tile([128, n_rb], F32, tag="dr")
            nc.gpsimd.dma_start(drt[:], dr_ext[:])

            for c in range(1, n_xc):
                nc.sync.dma_start(
                    xsr[c][:], xs_ext[:, c * XC * in_f:(c + 1) * XC * in_f]
                )

            def xs_fetch(j):
                c, kk = j // XC, j % XC
                return xsr[c][:, kk * in_f:(kk + 1) * in_f]

            def hs_fetch(j):
                # stream gathered hidden features in exact consumption order
                kk, i = j // N_CORES, j % N_CORES
                t = hstream.tile([128, hid], BF16, tag="hsgs", name=f"hsgs_{j}")
                nc.sync.dma_start(
                    t[:],
                    hs_gath[0][i * 128:(i + 1) * 128, kk * hid:(kk + 1) * hid],
                )
                return t

            def aggregate(src_fetch, n_f, label):
                """P_T[f, r] = sum_n src[n, f] * A~[r, n], feature-major psum."""
                acc = [
                    psum.tile([128, rw], F32, tag="acc", name=f"acc_{label}_{i}")
                    for i in range(n_f * n_rh)
                ]
                for g in range(n_g):
                    atq = stream.tile([128, KB * rows], BF16, tag="atq",
                                      name=f"atq_{label}_{g}")
                    nc.sync.dma_start(
                        atq[:], at_ext[:, g * KB * rows:(g + 1) * KB * rows]
                    )
                    for kk in range(KB):
                        j = g * KB + kk
                        src = src_fetch(j)
                        for f in range(n_f):
                            for rh in range(n_rh):
                                nc.tensor.matmul(
                                    acc[f * n_rh + rh][:],
                                    src[:, f * 128:(f + 1) * 128],
                                    atq[:, kk * rows + rh * rw:
                                        kk * rows + (rh + 1) * rw],
                                    start=(j == 0),
                                    stop=(j == n_k - 1),
                                )
                # drain feature-major accumulation to SBUF (cast bf16)
                ps = []
                for f in range(n_f):
                    t = feat.tile([128, rows], BF16, tag="ps", name=f"ps_{label}_{f}")
                    for rh in range(n_rh):
                        nc.vector.tensor_copy(
                            t[:, rh * rw:(rh + 1) * rw], acc[f * n_rh + rh][:]
                        )
                    ps.append(t)
                return ps

            def fire_allgather(s):
                nc.gpsimd.collective_compute(
                    "AllGather",
                    mybir.AluOpType.bypass,
                    replica_groups=[list(range(N_CORES))],
                    ins=[hs_loc[s][:]],
                    outs=[hs_gath[s][:]],
                )

            # ---- layer 1 ----
            p1s = aggregate(xs_fetch, n_fi, "agg1")
            for rb in range(n_rb):
                zp = psum.tile([128, hid], F32, tag="acc")
                for fc in range(n_fi):
                    nc.tensor.matmul(
                        zp[:],
                        p1s[fc][:, rb * 128:(rb + 1) * 128],
                        w1t[fc][:],
                        start=(fc == 0),
                        stop=(fc == n_fi - 1),
                    )
                v = ep.tile([128, hid], F32, tag="v1")
                nc.vector.tensor_scalar_mul(v[:], zp[:], drt[:, rb:rb + 1])
                v2 = ep.tile([128, hid], F32, tag="v2")
                nc.vector.tensor_add(v2[:], v[:], b1t[:])
                hst = ep.tile([128, hid], BF16, tag="hst")
                nc.scalar.activation(
                    hst[:], v2[:], mybir.ActivationFunctionType.Relu,
                    scale=drt[:, rb:rb + 1],
                )
                s = 0 if (n_splits == 1 or rb < half) else 1
                rb_s = rb if s == 0 else rb - half
                nc.sync.dma_start(
                    hs_loc[s][:, rb_s * hid:(rb_s + 1) * hid], hst[:]
                )
                if n_splits == 2 and rb == half - 1:
                    fire_allgather(0)
            fire_allgather(1 if n_splits == 2 else 0)

            # ---- layer 2 ----
            p2s = aggregate(hs_fetch, n_fh, "agg2")
            for rb in range(n_rb):
                zp = psum.tile([128, out_f], F32, tag="acc")
                for fc in range(n_fh):
                    nc.tensor.matmul(
                        zp[:],
                        p2s[fc][:, rb * 128:(rb + 1) * 128],
                        w2t[fc][:],
                        start=(fc == 0),
                        stop=(fc == n_fh - 1),
                    )
                v = ep.tile([128, out_f], F32, tag="vo1")
                nc.vector.tensor_scalar_mul(v[:], zp[:], drt[:, rb:rb + 1])
                o = ep.tile([128, out_f], F32, tag="vo2")
                nc.vector.tensor_add(o[:], v[:], b2t[:])
                nc.sync.dma_start(out_ext[rb * 128:(rb + 1) * 128, :], o[:])

    # drop the implicit kernel-entry barrier collective (~40us): the
    # mid-kernel AllGathers provide all the cross-core sync the math needs.
    nc._bir_kernel_barrier_sem_replica_groups = []
    nc.finalize()
    return nc


def _to_partition_major(a, n_k, order=None):
    """[n_k*128, F] row-major -> [128, n_k*F], chunk order[j] at column j*F."""
    f = a.shape[1]
    b = a.reshape(n_k, 128, f)
    if order is not None:
        b = b[order]
    return np.ascontiguousarray(b.transpose(1, 0, 2).reshape(128, n_k * f))


def prep_inputs(x, edge_index, W1, b1, W2, b2):
    """Host-side prep: dense normalized adjacency + per-core shards."""
    x = np.asarray(x, dtype=np.float32)
    edge_index = np.asarray(edge_index)
    W1 = np.asarray(W1, dtype=np.float32)
    b1 = np.asarray(b1, dtype=np.float32)
    W2 = np.asarray(W2, dtype=np.float32)
    b2 = np.asarray(b2, dtype=np.float32)

    n = x.shape[0]
    rows = n // N_CORES
    n_rb = rows // 128
    n_k = n // 128
    order = _k_order(n_k, n_rb)

    adj = np.zeros((n, n), dtype=np.float32)
    adj[edge_index[0], edge_index[1]] = 1.0
    idx = np.arange(n)
    adj[idx, idx] += 1.0
    deg = np.maximum(adj.sum(axis=1), 1.0)
    dinv = (deg ** -0.5).astype(np.float32)

    xs = _to_partition_major(
        (x * dinv[:, None]).astype(ml_dtypes.bfloat16), n_k, order
    )
    w1b = W1.astype(ml_dtypes.bfloat16)
    w2b = W2.astype(ml_dtypes.bfloat16)
    b1bc = np.ascontiguousarray(np.broadcast_to(b1, (128, b1.shape[0]))).astype(np.float32)
    b2bc = np.ascontiguousarray(np.broadcast_to(b2, (128, b2.shape[0]))).astype(np.float32)

    in_maps = []
    for i in range(N_CORES):
        sl = slice(i * rows, (i + 1) * rows)
        ati = np.ascontiguousarray(adj[sl, :].T).astype(ml_dtypes.bfloat16)
        in_maps.append({
            "at": _to_partition_major(ati, n_k, order),
            "xs": xs,
            "w1": w1b,
            "w2": w2b,
            "b1bc": b1bc,
            "b2bc": b2bc,
            "dr8": np.ascontiguousarray(dinv[sl].reshape(n_rb, 128).T),
        })
    return in_maps


def kernel(x, edge_index, W1, b1, W2, b2):
    global LAST_RESULT
    x = np.asarray(x)
    n, in_f = x.shape
    hid = np.asarray(W1).shape[1]
    out_f = np.asarray(W2).shape[1]

    key = (n, in_f, hid, out_f)
    if key not in _NC_CACHE:
        _NC_CACHE[key] = build_gcn(n, in_f, hid, out_f)
    nc = _NC_CACHE[key]

    in_maps = prep_inputs(x, edge_index, W1, b1, W2, b2)
    res = run_bass_kernel_spmd(nc, in_maps, core_ids=list(range(N_CORES)))
    LAST_RESULT = res
    return np.concatenate([res.results[i]["out"] for i in range(N_CORES)], axis=0)
